# revision 7
# baseline (speedup 1.0000x reference)
"""Distributed GQA attention (llama-style RoPE) for one TRN2 chip (8 NeuronCores).

Sharding: core c handles batch b=c//4 and sequence-quarter q=c%4 (512 q-rows).
Each core projects Q for its own rows (all 32 heads), projects K/V for its own
512 positions, AllGathers K/V within its 4-core batch group, runs attention for
its rows, and applies the output projection. Output rows are disjoint across
cores, so no all-reduce is needed; the host concatenates.

On-chip dataflow (per core):
  xT   = x.T via PE transposes                     [d, rows]   f32
  qT   = wq.T @ xT   (f32r matmuls) -> RoPE -> bf16 [feat, sq]
  kT   = wk.T @ xT   -> RoPE -> bf16 -> AllGather   [feat, skv]
  v    = xT.T @ wv   -> bf16 -> AllGather           [skv, feat]
  sT   = kT_h.T @ qT_h  per head (row-packed pairs) [skv, sq]  psum f32
  e    = exp(sT/8) on ScalarE -> bf16
  oT   = [v|1].T @ e  accumulated over skv chunks   [65, sq]   psum f32
  oT_n = oT[:64] * recip(oT[64])                    -> f32 sbuf
  out  = oT_n.T @ wo  (f32r), 4 head-group partials summed in DRAM via DMA-CCE

RoPE uses the half-rotated layout: wq/wk columns are permuted on the HOST so
each head's features are [evens(32) | odds(32)]; cos/sin tables are shipped
pre-transposed/tiled as CC/SS [128, sq].
"""
import sys

sys.path.insert(0, "/opt/trn_rl_repo")

import numpy as np
from contextlib import ExitStack

import concourse.bass as bass
import concourse.mybir as mybir
import concourse.tile as tile
from concourse import bacc
from concourse.bass_utils import run_bass_kernel_spmd
from concourse.masks import make_identity

B, S, D = 2, 2048, 2048
NQ, NKV, HD = 32, 8, 64
NCORES = 8
GPB = 4                 # cores per batch group
SQ = S // GPB           # 512 q-rows per core
P = 128
DC = D // P             # 16 contraction chunks
KF = NKV * HD           # 512 kv feature dim
KFC = KF // P           # 4 kv feature chunks
SC = S // P             # 16 skv chunks
RQ = SQ // P            # 4 q-row blocks

FP = mybir.dt.float32
BF = mybir.dt.bfloat16
F32R = mybir.dt.float32r
EXPF = mybir.ActivationFunctionType.Exp
EXP_SCALE = 1.0 / 8.0   # 1/sqrt(HD)


def _r(ap):
    return ap.bitcast(F32R)


def build():
    nc = bacc.Bacc("TRN2", target_bir_lowering=False, debug=False,
                   num_devices=NCORES)

    x_e = nc.dram_tensor("x", [SQ, D], FP, kind="ExternalInput").ap()
    wq_e = nc.dram_tensor("wq", [D, D], FP, kind="ExternalInput").ap()
    wk_e = nc.dram_tensor("wk", [D, KF], FP, kind="ExternalInput").ap()
    wv_e = nc.dram_tensor("wv", [D, KF], FP, kind="ExternalInput").ap()
    wo_e = nc.dram_tensor("wo", [D, D], FP, kind="ExternalInput").ap()
    cc_e = nc.dram_tensor("cc", [P, SQ], FP, kind="ExternalInput").ap()
    ss_e = nc.dram_tensor("ss", [P, SQ], FP, kind="ExternalInput").ap()
    out_e = nc.dram_tensor("out", [SQ, D], FP, kind="ExternalOutput").ap()

    groups = [[0, 1, 2, 3], [4, 5, 6, 7]]

    with tile.TileContext(nc) as tc, ExitStack() as ctx:
        sb = ctx.enter_context(tc.tile_pool(name="sb", bufs=1))
        rp = ctx.enter_context(tc.tile_pool(name="rp", bufs=2))
        epool = ctx.enter_context(tc.tile_pool(name="epool", bufs=2))
        npool = ctx.enter_context(tc.tile_pool(name="npool", bufs=2))
        opool = ctx.enter_context(tc.tile_pool(name="opool", bufs=2))
        stgp = ctx.enter_context(tc.tile_pool(name="stgp", bufs=3))
        dram = ctx.enter_context(tc.tile_pool(name="dram", bufs=1, space="DRAM"))
        pp = ctx.enter_context(tc.tile_pool(name="pp", bufs=2, space="PSUM"))
        psc = ctx.enter_context(tc.tile_pool(name="psc", bufs=2, space="PSUM"))
        po = ctx.enter_context(tc.tile_pool(name="po", bufs=2, space="PSUM"))

        # ---- constants ----
        cc_sb = sb.tile([P, SQ], FP)
        ss_sb = sb.tile([P, SQ], FP)
        ident = sb.tile([P, P], FP)
        nc.sync.dma_start(cc_sb[:], cc_e)
        nc.sync.dma_start(ss_sb[:], ss_e)
        make_identity(nc, ident[:])

        def rope_chunk(ps, dst):
            """dst = RoPE(ps) in half-rotated layout; ps [128,SQ] psum f32."""
            t0 = rp.tile([P, SQ], FP, tag="t0")
            t1 = rp.tile([P, SQ], FP, tag="t1")
            nc.vector.tensor_mul(t0[:], ps[:], cc_sb[:])
            for blk in range(4):
                o0, i0 = blk * 32, (blk ^ 1) * 32
                nc.vector.tensor_mul(t1[o0:o0 + 32, :], ps[i0:i0 + 32, :],
                                     ss_sb[o0:o0 + 32, :])
            nc.vector.tensor_add(dst, t0[:], t1[:])

        qT = sb.tile([P, DC, SQ], BF)
        kT_own = sb.tile([P, KFC, SQ], BF)
        v_own = sb.tile([P, RQ, KF], BF)
        kag_in = dram.tile([KF, SQ], BF)
        kag_out = dram.tile([GPB * KF, SQ], BF)
        vag_in = dram.tile([SQ, KF], BF)
        vag_out = dram.tile([S, KF], BF)
        out_dram = dram.tile([SQ, D], FP)

        with tc.tile_pool(name="early", bufs=1) as early, \
             tc.tile_pool(name="wqp", bufs=2) as wqp:
            # ---- load x, build xT via PE transposes ----
            x_sb = early.tile([P, RQ, D], FP, tag="big32", name="x_sb")
            nc.sync.dma_start(x_sb[:], x_e.rearrange("(c p) d -> p c d", p=P))
            xT = early.tile([P, DC, SQ], F32R, tag="xT", name="xT")
            for rc in range(RQ):
                for dc in range(DC):
                    tp = pp.tile([P, 512], FP, tag="pp", name="tp")
                    nc.tensor.transpose(tp[:, :P], x_sb[:, rc, dc * P:(dc + 1) * P],
                                        ident[:])
                    nc.vector.tensor_copy(xT[:, dc, rc * P:(rc + 1) * P], tp[:, :P])

            # ---- K projection + RoPE -> AllGather ----
            wk_sb = early.tile([P, DC, KF], F32R, tag="big32", name="wk_sb")
            nc.sync.dma_start(wk_sb[:], wk_e.rearrange("(o p) f -> p o f", p=P).bitcast(F32R))
            for fc in range(KFC):
                ps = pp.tile([P, 512], FP, tag="pp", name="kps")
                for dc in range(DC):
                    nc.tensor.matmul(ps[:, :SQ],
                                     lhsT=wk_sb[:, dc, fc * P:(fc + 1) * P],
                                     rhs=xT[:, dc, :],
                                     start=(dc == 0), stop=(dc == DC - 1))
                rope_chunk(ps[:, :SQ], kT_own[:, fc, :])
            nc.sync.dma_start(kag_in[:].rearrange("(c p) s -> p c s", p=P),
                              kT_own[:])
            nc.gpsimd.collective_compute(
                "AllGather", mybir.AluOpType.bypass, replica_groups=groups,
                ins=[kag_in[:]], outs=[kag_out[:]])

            # ---- V projection -> AllGather ----
            wv_sb = early.tile([P, DC, KF], F32R, tag="big32", name="wv_sb")
            nc.sync.dma_start(wv_sb[:], wv_e.rearrange("(o p) f -> p o f", p=P).bitcast(F32R))
            for pc in range(RQ):
                ps = pp.tile([P, 512], FP, tag="pp", name="vps")
                for dc in range(DC):
                    nc.tensor.matmul(ps[:, :KF],
                                     lhsT=xT[:, dc, pc * P:(pc + 1) * P],
                                     rhs=wv_sb[:, dc, :],
                                     start=(dc == 0), stop=(dc == DC - 1))
                nc.vector.tensor_copy(v_own[:, pc, :], ps[:, :KF])
            nc.sync.dma_start(vag_in[:].rearrange("(c p) f -> p c f", p=P),
                              v_own[:])
            nc.gpsimd.collective_compute(
                "AllGather", mybir.AluOpType.bypass, replica_groups=groups,
                ins=[vag_in[:]], outs=[vag_out[:]])

            # ---- Q projection + RoPE (16 chunks of 128 feats) ----
            for qc in range(DC):
                wq_sb = wqp.tile([P, DC, P], F32R, tag="wq", name=f"wq_{qc}")
                nc.sync.dma_start(
                    wq_sb[:],
                    wq_e[:, qc * P:(qc + 1) * P].rearrange("(o p) f -> p o f", p=P).bitcast(F32R))
                ps = pp.tile([P, 512], FP, tag="pp", name="qps")
                for dc in range(DC):
                    nc.tensor.matmul(ps[:, :SQ], lhsT=wq_sb[:, dc, :],
                                     rhs=xT[:, dc, :],
                                     start=(dc == 0), stop=(dc == DC - 1))
                rope_chunk(ps[:, :SQ], qT[:, qc, :])

        # ---- land gathered K/V ----
        kT = sb.tile([P, KFC, S], BF)
        for fc in range(KFC):
            for r in range(GPB):
                nc.sync.dma_start(kT[:, fc, r * SQ:(r + 1) * SQ],
                                  kag_out[r * KF + fc * P: r * KF + (fc + 1) * P, :])
        v_aug = sb.tile([P, NKV, SC, HD + 1], BF)
        nc.gpsimd.memset(v_aug[:], 1.0)
        for c in range(SC):
            nc.sync.dma_start(
                v_aug[:, :, c, 0:HD],
                vag_out[c * P:(c + 1) * P, :].rearrange("p (kv d) -> p kv d", d=HD))

        # ---- attention + interleaved out-projection ----
        oT = sb.tile([P, RQ, SQ], F32R)

        for g in range(4):                    # 4 groups of 4 pairs
            for pi in range(4):               # pairs within group
                pair = g * 4 + pi
                kc = pair % 4                 # kv chunk holding both kv heads
                kva, kvb = 2 * (pair % 4), 2 * (pair % 4) + 1
                psOA = po.tile([P, 512], FP, tag="po", name="psOA")
                psOB = po.tile([P, 512], FP, tag="po", name="psOB")
                for qtr in range(4):          # 4 skv chunks per exp tile
                    expA = epool.tile([P, 4, SQ], BF, tag="expA", name="expA")
                    expB = epool.tile([P, 4, SQ], BF, tag="expB", name="expB")
                    for cg in range(2):       # two skv chunks per psum tile
                        psA = psc.tile([P, 1024], FP, tag="psc", name="psA")
                        psB = psc.tile([P, 1024], FP, tag="psc", name="psB")
                        for h in range(2):
                            c = qtr * 4 + 2 * cg + h
                            nc.tensor.matmul(psA[:, h * SQ:(h + 1) * SQ],
                                             lhsT=kT[0:64, kc, c * P:(c + 1) * P],
                                             rhs=qT[0:64, pair, :],
                                             start=True, stop=True,
                                             tile_position=(0, 0))
                            nc.tensor.matmul(psB[:, h * SQ:(h + 1) * SQ],
                                             lhsT=kT[64:128, kc, c * P:(c + 1) * P],
                                             rhs=qT[64:128, pair, :],
                                             start=True, stop=True,
                                             tile_position=(64, 0))
                        nc.scalar.activation(expA[:, 2 * cg:2 * cg + 2, :], psA[:],
                                             EXPF, scale=EXP_SCALE)
                        nc.scalar.activation(expB[:, 2 * cg:2 * cg + 2, :], psB[:],
                                             EXPF, scale=EXP_SCALE)
                    for cq in range(4):
                        c = qtr * 4 + cq
                        nc.tensor.matmul(psOA[:HD + 1, :SQ],
                                         lhsT=v_aug[:, kva, c, :],
                                         rhs=expA[:, cq, :],
                                         start=(c == 0), stop=(c == SC - 1))
                        nc.tensor.matmul(psOB[:HD + 1, :SQ],
                                         lhsT=v_aug[:, kvb, c, :],
                                         rhs=expB[:, cq, :],
                                         start=(c == 0), stop=(c == SC - 1))
                for hp, psO, dst0 in ((0, psOA, 0), (1, psOB, 64)):
                    rd = npool.tile([1, SQ], FP, tag="rd", name="rd")
                    nc.vector.reciprocal(rd[:], psO[HD:HD + 1, :SQ])
                    rbc = npool.tile([HD, SQ], FP, tag="rbc", name="rbc")
                    nc.gpsimd.partition_broadcast(rbc[:], rd[:])
                    nc.vector.tensor_mul(oT[dst0:dst0 + HD, pi, :],
                                         psO[0:HD, :SQ], rbc[:])

            # out-projection partial for this group's 512 feature dims
            for nf in range(4):
                wo_nf = opool.tile([P, 4, 512], F32R, tag="wo", name="wo_nf")
                for ch in range(4):
                    nc.sync.dma_start(
                        wo_nf[:, ch, :],
                        wo_e[(g * 4 + ch) * P:(g * 4 + ch + 1) * P,
                             nf * 512:(nf + 1) * 512].bitcast(F32R))
                for m in range(RQ):
                    ps = pp.tile([P, 512], FP, tag="pp", name="ops")
                    for ch in range(4):
                        nc.tensor.matmul(ps[:],
                                         lhsT=oT[:, ch, m * P:(m + 1) * P],
                                         rhs=wo_nf[:, ch, :],
                                         start=(ch == 0), stop=(ch == 3))
                    stg = stgp.tile([P, 512], FP, tag="stg", name="stg")
                    nc.vector.tensor_copy(stg[:], ps[:])
                    dst = out_dram[m * P:(m + 1) * P, nf * 512:(nf + 1) * 512]
                    if g == 0:
                        nc.sync.dma_start(dst, stg[:])
                    else:
                        nc.gpsimd.dma_start(dst, stg[:],
                                            accum_op=mybir.AluOpType.add)

        nc.sync.dma_start(out_e, out_dram[:])

    nc.compile()
    return nc


_NC = None


def _get_nc():
    global _NC
    if _NC is None:
        _NC = build()
    return _NC


def _host_prep(inputs):
    """Permute wq/wk to half-rotated layout, build CC/SS tables, slice shards."""
    x = np.asarray(inputs["x"], np.float32)
    cos = np.asarray(inputs["cos"], np.float32)
    sin = np.asarray(inputs["sin"], np.float32)
    wq = np.asarray(inputs["wq"], np.float32)
    wk = np.asarray(inputs["wk"], np.float32)
    wv = np.ascontiguousarray(np.asarray(inputs["wv"], np.float32))
    wo = np.ascontiguousarray(np.asarray(inputs["wo"], np.float32))

    def perm_cols(w, nheads):
        idx = np.empty(nheads * HD, np.int64)
        for h in range(nheads):
            idx[h * HD:h * HD + 32] = h * HD + 2 * np.arange(32)
            idx[h * HD + 32:(h + 1) * HD] = h * HD + 2 * np.arange(32) + 1
        return np.ascontiguousarray(w[:, idx])

    wq_p = perm_cols(wq, NQ)
    wk_p = perm_cols(wk, NKV)
    cosT = np.ascontiguousarray(cos.T)            # [32, S]
    sinT = np.ascontiguousarray(sin.T)
    CC = np.tile(cosT, (4, 1))                    # [128, S]
    SS = np.concatenate([-sinT, sinT, -sinT, sinT], 0)

    in_maps = []
    for c in range(NCORES):
        b, q = c // GPB, c % GPB
        sl = slice(q * SQ, (q + 1) * SQ)
        in_maps.append({
            "x": np.ascontiguousarray(x[b, sl, :]),
            "wq": wq_p, "wk": wk_p, "wv": wv, "wo": wo,
            "cc": np.ascontiguousarray(CC[:, sl]),
            "ss": np.ascontiguousarray(SS[:, sl]),
        })
    return in_maps


def kernel(**inputs):
    nc = _get_nc()
    in_maps = _host_prep(inputs)
    res = run_bass_kernel_spmd(nc, in_maps, core_ids=list(range(NCORES)))
    out = np.empty((B, S, D), np.float32)
    for c in range(NCORES):
        b, q = c // GPB, c % GPB
        out[b, q * SQ:(q + 1) * SQ, :] = res.results[c]["out"]
    return out


# revision 16
# speedup vs baseline: 12116.1599x; 12116.1599x over previous
"""Distributed GQA attention (llama-style RoPE) for one TRN2 chip (8 NeuronCores).

Sharding: core c handles batch b=c//4 and sequence-quarter q=c%4 (512 q-rows).
Each core projects Q for its own rows (all 32 heads), projects K/V for its own
512 positions, AllGathers K/V within its 4-core batch group, runs attention for
its rows, and applies the output projection. Output rows are disjoint across
cores, so no all-reduce is needed; the host concatenates.

On-chip dataflow (per core):
  xT   = x.T via PE transposes                      [d, rows]   f32
  kT   = wk.T @ xT   -> RoPE -> bf16 -> AllGather   [feat, skv]
  v    = xT.T @ wv   -> bf16 -> AllGather           [skv, feat]
  per head pair (with the previous group's out-proj interleaved):
    qT   = wq.T @ xT (f32r) -> RoPE -> bf16         [feat, sq]
    sT   = kT_h.T @ qT_h  (row-packed pairs)        [skv, sq]  psum f32
    e    = exp(sT/8) on ScalarE -> bf16
    oT   = [v|1x32].T @ e  accum over skv chunks    [96, sq]   psum f32
           (cols 64:96 are ones -> denominator lands replicated 32-wide)
    oT_n = oT[:64] * recip(oT[64:96])               -> f32r sbuf
  out  = oT_n.T @ wo  (f32r), 4 head-group partials summed in DRAM via DMA-CCE

All weights are pre-swizzled on the HOST into the exact SBUF layouts so every
DMA is a fully-linear copy. RoPE uses the half-rotated layout: wq/wk columns
are permuted on the host so each head's features are [evens(32) | odds(32)];
cos/sin tables are shipped pre-transposed/tiled as CC/SS [128, sq].
"""
import sys

sys.path.insert(0, "/opt/trn_rl_repo")

import numpy as np
from contextlib import ExitStack

import concourse.bass as bass
import concourse.mybir as mybir
import concourse.tile as tile
from concourse import bacc
from concourse.bass_utils import run_bass_kernel_spmd
from concourse.masks import make_identity

B, S, D = 2, 2048, 2048
NQ, NKV, HD = 32, 8, 64
NCORES = 8
GPB = 4                 # cores per batch group
SQ = S // GPB           # 512 q-rows per core
P = 128
DC = D // P             # 16 contraction chunks
KF = NKV * HD           # 512 kv feature dim
KFC = KF // P           # 4 kv feature chunks
SC = S // P             # 16 skv chunks
RQ = SQ // P            # 4 q-row blocks
VW = HD + 32            # v_aug width: 64 v cols + 32 ones cols

FP = mybir.dt.float32
BF = mybir.dt.bfloat16
F32R = mybir.dt.float32r
EXPF = mybir.ActivationFunctionType.Exp
EXP_SCALE = 1.0 / 8.0   # 1/sqrt(HD)


def build(solo=False):
    nc = bacc.Bacc("TRN2", target_bir_lowering=False, debug=False,
                   num_devices=1 if solo else NCORES)

    x_e = nc.dram_tensor("x", [P, RQ, D], FP, kind="ExternalInput").ap()
    wq_e = nc.dram_tensor("wq", [DC, P, DC, P], FP, kind="ExternalInput").ap()
    wk_e = nc.dram_tensor("wk", [P, DC, KF], FP, kind="ExternalInput").ap()
    wv_e = nc.dram_tensor("wv", [P, DC, KF], FP, kind="ExternalInput").ap()
    wo_e = nc.dram_tensor("wo", [RQ, RQ, P, RQ, 512], FP, kind="ExternalInput").ap()
    cc_e = nc.dram_tensor("cc", [P, SQ], FP, kind="ExternalInput").ap()
    ss_e = nc.dram_tensor("ss", [P, SQ], FP, kind="ExternalInput").ap()
    out_e = nc.dram_tensor("out", [SQ, D], FP, kind="ExternalOutput").ap()

    groups = [[0, 1, 2, 3], [4, 5, 6, 7]]

    with tile.TileContext(nc) as tc, ExitStack() as ctx:
        sb = ctx.enter_context(tc.tile_pool(name="sb", bufs=1))
        rp = ctx.enter_context(tc.tile_pool(name="rp", bufs=2))
        epool = ctx.enter_context(tc.tile_pool(name="epool", bufs=2))
        npool = ctx.enter_context(tc.tile_pool(name="npool", bufs=2))
        opool = ctx.enter_context(tc.tile_pool(name="opool", bufs=2))
        otp = ctx.enter_context(tc.tile_pool(name="otp", bufs=2))
        stgp = ctx.enter_context(tc.tile_pool(name="stgp", bufs=2))
        early = ctx.enter_context(tc.tile_pool(name="early", bufs=1))
        wqp = ctx.enter_context(tc.tile_pool(name="wqp", bufs=2))
        dram = ctx.enter_context(tc.tile_pool(name="dram", bufs=1, space="DRAM"))
        pp = ctx.enter_context(tc.tile_pool(name="pp", bufs=2, space="PSUM"))
        psc = ctx.enter_context(tc.tile_pool(name="psc", bufs=2, space="PSUM"))
        po = ctx.enter_context(tc.tile_pool(name="po", bufs=2, space="PSUM"))

        # ---- constants ----
        cc_sb = sb.tile([P, SQ], FP)
        ss_sb = sb.tile([P, SQ], FP)
        ident = sb.tile([P, P], FP)
        nc.sync.dma_start(cc_sb[:], cc_e)
        nc.sync.dma_start(ss_sb[:], ss_e)
        make_identity(nc, ident[:])

        def rope_chunk(ps, dst):
            """dst = RoPE(ps) in half-rotated layout; ps [128,SQ] psum f32."""
            t0 = rp.tile([P, SQ], FP, tag="t0")
            t1 = rp.tile([P, SQ], FP, tag="t1")
            nc.vector.tensor_mul(t0[:], ps[:], cc_sb[:])
            for blk in range(4):
                o0, i0 = blk * 32, (blk ^ 1) * 32
                nc.vector.tensor_mul(t1[o0:o0 + 32, :], ps[i0:i0 + 32, :],
                                     ss_sb[o0:o0 + 32, :])
            nc.vector.tensor_add(dst, t0[:], t1[:])

        qT = sb.tile([P, DC, SQ], BF)
        kag_in = dram.tile([KF, SQ], BF)
        kag_out = dram.tile([GPB * KF, SQ], BF)
        vag_in = dram.tile([SQ, KF], BF)
        vag_out = dram.tile([S, KF], BF)
        out_dram = dram.tile([SQ, D], FP)

        # ---- pure input loads first (sync queue stays unblocked) ----
        x_sb = early.tile([P, RQ, D], FP, tag="big32", name="x_sb")
        for rc in range(RQ):
            nc.sync.dma_start(x_sb[:, rc, :], x_e[:, rc, :])
        wq_tiles = {}
        for pair in (0, 1):
            w = wqp.tile([P, DC, P], F32R, tag="wq", name=f"wq_{pair}")
            nc.sync.dma_start(w[:], wq_e[pair].bitcast(F32R))
            wq_tiles[pair] = w

        # ---- xT via PE transposes (dc-outer so k-proj can pipeline) ----
        xT = early.tile([P, DC, SQ], F32R, tag="xT", name="xT")
        for dc in range(DC):
            for rc in range(RQ):
                tp = pp.tile([P, 512], FP, tag="pp", name="tp")
                nc.tensor.transpose(tp[:, :P], x_sb[:, rc, dc * P:(dc + 1) * P],
                                    ident[:])
                nc.vector.tensor_copy(xT[:, dc, rc * P:(rc + 1) * P], tp[:, :P])

        def qproj(pair):
            if pair in wq_tiles:
                wq_sb = wq_tiles.pop(pair)
            else:
                wq_sb = wqp.tile([P, DC, P], F32R, tag="wq", name=f"wq_{pair}")
                nc.sync.dma_start(wq_sb[:], wq_e[pair].bitcast(F32R))
            qps = pp.tile([P, 512], FP, tag="pp", name="qps")
            for dc in range(DC):
                nc.tensor.matmul(qps[:, :SQ], lhsT=wq_sb[:, dc, :],
                                 rhs=xT[:, dc, :],
                                 start=(dc == 0), stop=(dc == DC - 1))
            rope_chunk(qps[:, :SQ], qT[:, pair, :])

        # q-projection for the first two pairs fills the wk-load bubble
        qproj(0)
        qproj(1)

        # ---- K projection + RoPE -> AllGather ----
        wk_sb = early.tile([P, DC, KF], F32R, tag="big32", name="wk_sb")
        nc.sync.dma_start(wk_sb[:], wk_e.bitcast(F32R))
        kT_own = sb.tile([P, KFC, SQ], BF, tag="own4", name="kT_own")
        for fc in range(KFC):
            ps = pp.tile([P, 512], FP, tag="pp", name="kps")
            for dc in range(DC):
                nc.tensor.matmul(ps[:, :SQ],
                                 lhsT=wk_sb[:, dc, fc * P:(fc + 1) * P],
                                 rhs=xT[:, dc, :],
                                 start=(dc == 0), stop=(dc == DC - 1))
            rope_chunk(ps[:, :SQ], kT_own[:, fc, :])
        nc.sync.dma_start(kag_in[:].rearrange("(c p) s -> p c s", p=P),
                          kT_own[:])
        if solo:
            for r in range(GPB):
                nc.sync.dma_start(kag_out[r * KF:(r + 1) * KF, :], kag_in[:])
        else:
            nc.gpsimd.collective_compute(
                "AllGather", mybir.AluOpType.bypass, replica_groups=groups,
                ins=[kag_in[:]], outs=[kag_out[:]])

        # q-projection for pairs 2/3 fills the wv-load bubble
        qproj(2)
        qproj(3)

        # ---- V projection -> AllGather ----
        # wv reuses the x slot (x is dead once transposes finished)
        wv_sb = early.tile([P, DC, KF], F32R, tag="big32", name="wv_sb")
        nc.sync.dma_start(wv_sb[:], wv_e.bitcast(F32R))
        v_own = sb.tile([P, RQ, KF], BF, tag="own4", name="v_own")
        for pc in range(RQ):
            ps = pp.tile([P, 512], FP, tag="pp", name="vps")
            for dc in range(DC):
                nc.tensor.matmul(ps[:, :KF],
                                 lhsT=xT[:, dc, pc * P:(pc + 1) * P],
                                 rhs=wv_sb[:, dc, :],
                                 start=(dc == 0), stop=(dc == DC - 1))
            nc.vector.tensor_copy(v_own[:, pc, :], ps[:, :KF])
        nc.sync.dma_start(vag_in[:].rearrange("(c p) f -> p c f", p=P),
                          v_own[:])
        if solo:
            for r in range(GPB):
                nc.sync.dma_start(vag_out[r * SQ:(r + 1) * SQ, :], vag_in[:])
        else:
            nc.gpsimd.collective_compute(
                "AllGather", mybir.AluOpType.bypass, replica_groups=groups,
                ins=[vag_in[:]], outs=[vag_out[:]])

        # ---- land gathered K/V ----
        kT = sb.tile([P, KFC, S], BF)
        for fc in range(KFC):
            for r in range(GPB):
                nc.scalar.dma_start(kT[:, fc, r * SQ:(r + 1) * SQ],
                                    kag_out[r * KF + fc * P: r * KF + (fc + 1) * P, :])
        v_aug = early.tile([P, NKV, SC, VW], BF, tag="big32", name="v_aug")
        nc.gpsimd.memset(v_aug[:], 1.0)
        for c in range(SC):
            nc.gpsimd.dma_start(
                v_aug[:, :, c, 0:HD],
                vag_out[c * P:(c + 1) * P, :].rearrange("p (kv d) -> p kv d", d=HD))

        # ---- per-pair: Q proj + attention; prev group's out-proj interleaved ----
        oT_tiles = {}

        def out_proj(g, nf):
            """Emit one nf-tile (512 out cols) of group g's out-projection."""
            oT = oT_tiles[g]
            wo_nf = opool.tile([P, 4, 512], F32R, tag="wo", name="wo_nf")
            nc.sync.dma_start(wo_nf[:], wo_e[g, nf].bitcast(F32R))
            for m in range(RQ):
                dst = out_dram[m * P:(m + 1) * P, nf * 512:(nf + 1) * 512]
                prev = None
                if g == 3:
                    prev = stgp.tile([P, 512], FP, tag="prev", name="prev")
                    nc.scalar.dma_start(prev[:], dst)
                ps = po.tile([P, 512], FP, tag="po", name="ops")
                for ch in range(4):
                    nc.tensor.matmul(ps[:],
                                     lhsT=oT[:, ch, m * P:(m + 1) * P],
                                     rhs=wo_nf[:, ch, :],
                                     start=(ch == 0), stop=(ch == 3))
                stg = stgp.tile([P, 512], FP, tag="stg", name="stg")
                if g == 3:
                    nc.vector.tensor_add(stg[:], ps[:], prev[:])
                    nc.sync.dma_start(
                        out_e[m * P:(m + 1) * P, nf * 512:(nf + 1) * 512], stg[:])
                elif g == 0:
                    nc.vector.tensor_copy(stg[:], ps[:])
                    nc.gpsimd.dma_start(dst, stg[:])
                else:
                    nc.vector.tensor_copy(stg[:], ps[:])
                    nc.gpsimd.dma_start(dst, stg[:], accum_op=mybir.AluOpType.add)

        for g in range(4):                    # 4 groups of 4 pairs
            oT_tiles[g] = otp.tile([P, RQ, SQ], F32R, tag="oT", name=f"oT_{g}")
            for pi in range(4):               # pairs within group
                pair = g * 4 + pi
                kc = pair % 4                 # kv chunk holding both kv heads
                kva, kvb = 2 * (pair % 4), 2 * (pair % 4) + 1

                # Q projection + RoPE for this pair's 128 features
                if pair >= 4:
                    qproj(pair)

                psOA = po.tile([P, 512], FP, tag="po", name="psOA")
                psOB = po.tile([P, 512], FP, tag="po", name="psOB")
                for qtr in range(4):          # 4 skv chunks per exp tile
                    expA = epool.tile([P, 4, SQ], BF, tag="expA", name="expA")
                    expB = epool.tile([P, 4, SQ], BF, tag="expB", name="expB")
                    for cg in range(2):
                        psA = psc.tile([P, 1024], FP, tag="psc", name="psA")
                        psB = psc.tile([P, 1024], FP, tag="psc", name="psB")
                        for h in range(2):
                            c = qtr * 4 + 2 * cg + h
                            nc.tensor.matmul(psA[:, h * SQ:(h + 1) * SQ],
                                             lhsT=kT[0:64, kc, c * P:(c + 1) * P],
                                             rhs=qT[0:64, pair, :],
                                             start=True, stop=True,
                                             tile_position=(0, 0))
                            nc.tensor.matmul(psB[:, h * SQ:(h + 1) * SQ],
                                             lhsT=kT[64:128, kc, c * P:(c + 1) * P],
                                             rhs=qT[64:128, pair, :],
                                             start=True, stop=True,
                                             tile_position=(64, 0))
                        nc.scalar.activation(expA[:, 2 * cg:2 * cg + 2, :], psA[:],
                                             EXPF, scale=EXP_SCALE)
                        nc.scalar.activation(expB[:, 2 * cg:2 * cg + 2, :], psB[:],
                                             EXPF, scale=EXP_SCALE)
                    for cq in range(4):
                        c = qtr * 4 + cq
                        nc.tensor.matmul(psOA[:VW, :SQ],
                                         lhsT=v_aug[:, kva, c, :],
                                         rhs=expA[:, cq, :],
                                         start=(c == 0), stop=(c == SC - 1))
                        nc.tensor.matmul(psOB[:VW, :SQ],
                                         lhsT=v_aug[:, kvb, c, :],
                                         rhs=expB[:, cq, :],
                                         start=(c == 0), stop=(c == SC - 1))
                oT = oT_tiles[g]
                for psO, dst0 in ((psOA, 0), (psOB, 64)):
                    rbc = npool.tile([32, SQ], FP, tag="rbc", name="rbc")
                    nc.vector.reciprocal(rbc[:], psO[HD:VW, :SQ])
                    nc.vector.tensor_mul(oT[dst0:dst0 + 32, pi, :],
                                         psO[0:32, :SQ], rbc[:])
                    nc.vector.tensor_mul(oT[dst0 + 32:dst0 + 64, pi, :],
                                         psO[32:64, :SQ], rbc[:])

                if g >= 1:
                    out_proj(g - 1, pi)
            if g == 3:
                for nf in range(4):
                    out_proj(3, nf)

    nc.compile()
    return nc


_NC = None


def _get_nc():
    global _NC
    if _NC is None:
        _NC = build()
    return _NC


def _host_prep(inputs):
    """Permute wq/wk to half-rotated layout, swizzle all weights into the
    on-chip layouts (so device DMAs are linear), build CC/SS tables, slice
    per-core shards."""
    x = np.asarray(inputs["x"], np.float32)
    cos = np.asarray(inputs["cos"], np.float32)
    sin = np.asarray(inputs["sin"], np.float32)
    wq = np.asarray(inputs["wq"], np.float32)
    wk = np.asarray(inputs["wk"], np.float32)
    wv = np.asarray(inputs["wv"], np.float32)
    wo = np.asarray(inputs["wo"], np.float32)

    def perm_cols(w, nheads):
        idx = np.empty(nheads * HD, np.int64)
        for h in range(nheads):
            idx[h * HD:h * HD + 32] = h * HD + 2 * np.arange(32)
            idx[h * HD + 32:(h + 1) * HD] = h * HD + 2 * np.arange(32) + 1
        return np.ascontiguousarray(w[:, idx])

    wq_p = perm_cols(wq, NQ)
    wk_p = perm_cols(wk, NKV)
    # device layouts
    wq_dev = np.ascontiguousarray(
        wq_p.reshape(DC, P, DC, P).transpose(2, 1, 0, 3))      # [qc, p, o, f]
    wk_dev = np.ascontiguousarray(
        wk_p.reshape(DC, P, KF).transpose(1, 0, 2))            # [p, o, f]
    wv_dev = np.ascontiguousarray(
        wv.reshape(DC, P, KF).transpose(1, 0, 2))
    wo_dev = np.ascontiguousarray(
        wo.reshape(RQ, RQ, P, RQ, 512).transpose(0, 3, 2, 1, 4))  # [g,nf,p,ch,j]

    cosT = np.ascontiguousarray(cos.T)            # [32, S]
    sinT = np.ascontiguousarray(sin.T)
    CC = np.tile(cosT, (4, 1))                    # [128, S]
    SS = np.concatenate([-sinT, sinT, -sinT, sinT], 0)

    in_maps = []
    for c in range(NCORES):
        b, q = c // GPB, c % GPB
        sl = slice(q * SQ, (q + 1) * SQ)
        x_dev = np.ascontiguousarray(
            x[b, sl, :].reshape(RQ, P, D).transpose(1, 0, 2))  # [p, rc, d]
        in_maps.append({
            "x": x_dev,
            "wq": wq_dev, "wk": wk_dev, "wv": wv_dev, "wo": wo_dev,
            "cc": np.ascontiguousarray(CC[:, sl]),
            "ss": np.ascontiguousarray(SS[:, sl]),
        })
    return in_maps


def kernel(**inputs):
    nc = _get_nc()
    in_maps = _host_prep(inputs)
    res = run_bass_kernel_spmd(nc, in_maps, core_ids=list(range(NCORES)))
    out = np.empty((B, S, D), np.float32)
    for c in range(NCORES):
        b, q = c // GPB, c % GPB
        out[b, q * SQ:(q + 1) * SQ, :] = res.results[c]["out"]
    return out
